# revision 22
# baseline (speedup 1.0000x reference)
"""Trainium2 Bass kernel for EntityMarker segment-reduce (span means).

Problem: sequence_output [128, 2048, 768] f32, entity_positions [128, 4] int.
For each batch b, compute the mean of sequence_output[b, s:e+1, :] for the
head span (cols 0,1) and tail span (cols 2,3), clamped like the reference.
Output: (head [128, 768], tail [128, 768]) f32.

v3.1 strategy (host-permuted bf16/fp8 triple stream, 8 cores):
  - Host computes clamped spans; per batch the union of the two spans is 1-2
    contiguous runs, split into subsegments of constant (head, tail)
    membership. Batches go to cores by greedy LPT on union size (16/core).
  - The host permutes the needed rows into the exact order the device
    consumes them, so the device does NO gather — every chunk is one plain
    2D DMA from a contiguous DRAM region. The DMA engines are elementwise
    (SBUF-write-side) bound, so bytes are minimized by SBUF dtype tiering
    on the min span count a row feeds (span-mean quantization error shrinks
    as 1/sqrt(count)):
      count < 16        -> bf16   (Z chunks)
      16 <= count < 64  -> fp8 e3m4 (D chunks)
      count >= 64       -> fp8 e4m3 (P chunks)
  - Chunk kinds (tile = [<=128 partitions, 4*768]):
      Z (bf16): partition = 2 two-row windows; 2 DVE adds (2x mode) ->
        bf16 reduced rows; 2 PE matmul pairs (bf16 1/count weights) -> PSUM1.
      D (e3m4): same shape; 2 DVE adds (1x) -> e3m4 reduced rows; 2 matmul
        pairs (bf16 lhsT x fp8 rhs runs at fp8 rate) -> PSUM1.
      P (e4m3): no DVE; 2 DoubleRow matmul pairs (0.5 cycles/row) contract
        256 raw rows each with 0/1 e4m3 per-row weights -> PSUM2.
      P1 (e4m3): single-row tail chunk, one plain pair -> PSUM2.
    Odd leftover bf16 rows are duplicated into a two-row window with halved
    weights; odd fp8 rows go to the P lane (per-row weights).
  - PSUM2 holds RAW per-segment sums; it is drained to a second output and
    the host folds final = out + out2 / count. This keeps P-lane weights
    exactly representable in e4m3.
  - The stream ends with a tiny P1 chunk so the only work after the last
    byte is one matmul pair + the PSUM drain.
  - The device program is UNIFORM across cores (SPMD); per-core shortfalls
    are padded with zero rows and zero weights.
"""

import os

import numpy as np

_B, _L, _H = 128, 2048, 768
_NCORES = 8
_BPC = _B // _NCORES  # batches per core
_SEG = 2 * _BPC       # segments per core: 16 head + 16 tail
_TMIN = int(os.environ.get("KERNEL_TMIN", "16"))   # min count for fp8
_T2 = int(os.environ.get("KERNEL_T2", "64"))       # min count for e4m3
_PSDMA = os.environ.get("KERNEL_PSDMA", "0") == "1"
_DBUFS = int(os.environ.get("KERNEL_DBUFS", "6"))
_PBUFS = int(os.environ.get("KERNEL_PBUFS", "6"))

_prog_cache = {}


def _mult16(n):
    return (n + 15) // 16 * 16


def _chunks_of(n_units, units_per_chunk):
    """Split n_units windows/rows into chunk partition-widths (mult of 16)."""
    full = 128 * units_per_chunk
    sizes = []
    left = n_units
    while left >= full:
        sizes.append(128)
        left -= full
    if left > 0:
        sizes.append(_mult16((left + units_per_chunk - 1) // units_per_chunk))
    return sizes


def _build_spec(nz, nd, np_rows):
    """Uniform chunk spec: list of (kind, partition_width).

    P/D interleaved (PE is the longest compute pole, feed it first and
    steadily), Z in the middle, ending with a 1-row-deep P1 chunk."""
    zc = _chunks_of(nz, 2)
    dc = _chunks_of(nd, 2)
    p_tail = min(np_rows, 128)
    pc = _chunks_of(np_rows - p_tail, 4)
    spec = []
    di, pi, zi = 0, 0, 0
    # round-robin: 3 P : 1 D : occasionally Z
    while di < len(dc) or pi < len(pc) or zi < len(zc):
        for _ in range(3):
            if pi < len(pc):
                spec.append(("P", pc[pi]))
                pi += 1
        if di < len(dc):
            spec.append(("D", dc[di]))
            di += 1
        if zi < len(zc):
            spec.append(("Z", zc[zi]))
            zi += 1
    spec.append(("P1", _mult16(p_tail)))
    return spec


def _pairs_of(kind):
    # (psum1 pairs, psum2 pairs) per chunk
    return {"Z": (2, 0), "D": (2, 0), "P": (0, 2), "P1": (0, 1)}[kind]


def _rows_per_part(kind):
    return {"Z": 4, "D": 4, "P": 4, "P1": 1}[kind]


def _build_program(nz, nd, np_rows):
    import concourse.mybir as mybir
    from concourse import bacc, tile

    f32 = mybir.dt.float32
    bf16 = mybir.dt.bfloat16
    e3 = mybir.dt.float8e3
    e4 = mybir.dt.float8e4

    spec = _build_spec(nz, nd, np_rows)
    n1 = sum(_pairs_of(k)[0] for k, _ in spec)
    n2 = sum(_pairs_of(k)[1] for k, _ in spec)
    rz = sum(4 * s for k, s in spec if k == "Z")
    rd = sum(4 * s for k, s in spec if k == "D")
    rp = sum(_rows_per_part(k) * s for k, s in spec if k in ("P", "P1"))

    nc = bacc.Bacc(None, target_bir_lowering=False)
    xz = nc.declare_dram_parameter("xz", [max(rz, 4), _H], bf16,
                                   isOutput=False)
    xd = nc.declare_dram_parameter("xd", [max(rd, 4), _H], e3,
                                   isOutput=False)
    xp = nc.declare_dram_parameter("xp", [max(rp, 4), _H], e4,
                                   isOutput=False)
    w = nc.declare_dram_parameter("w", [128, max(n1, 1) * _SEG], bf16,
                                  isOutput=False)
    # P-lane 0/1 weights: 64 cols per DR pair group-block (2x32), 32 for P1
    w8 = nc.declare_dram_parameter("w8", [128, max(n2, 1) * 2 * _SEG], e4,
                                   isOutput=False)
    # single fused output: cols [0:768] = weighted sums, [768:1536] = raw
    # P-lane sums (host divides by count and adds)
    outx = nc.declare_dram_parameter("outx", [_SEG, 2 * _H], f32,
                                     isOutput=True)

    with tile.TileContext(nc) as tc:
        with (
            tc.tile_pool(name="const", bufs=1) as cpool,
            tc.tile_pool(name="gz", bufs=2) as zpool,
            tc.tile_pool(name="gd", bufs=_DBUFS) as dpool,
            tc.tile_pool(name="gp", bufs=_PBUFS) as ppool_g,
            tc.tile_pool(name="red", bufs=8) as rpool,
            tc.tile_pool(name="psum", bufs=1, space="PSUM") as pspool,
        ):
            w_t = cpool.tile([128, max(n1, 1) * _SEG], bf16)
            nc.scalar.dma_start(out=w_t[:], in_=w[:])
            w8_t = cpool.tile([128, max(n2, 1) * 2 * _SEG], e4)
            nc.scalar.dma_start(out=w8_t[:], in_=w8[:])

            ps_a = pspool.tile([_SEG, 512], f32)
            ps_b = pspool.tile([_SEG, 256], f32)
            ps2_a = pspool.tile([_SEG, 512], f32)
            ps2_b = pspool.tile([_SEG, 256], f32)

            i1, i2 = [0], [0]

            def mm_pair(p, rhs):
                j = i1[0]
                lhsT = w_t[:p, j * _SEG:(j + 1) * _SEG]
                st = j == 0
                sp = j == n1 - 1
                i1[0] += 1
                nc.tensor.matmul(ps_a[:], lhsT, rhs[:p, 0:512],
                                 start=st, stop=sp)
                nc.tensor.matmul(ps_b[:], lhsT, rhs[:p, 512:_H],
                                 start=st, stop=sp)

            def mm_dr(p, g, goff):
                # DoubleRow pair: blocked APs [p, 2, N]; contracts 2*p rows
                j = i2[0]
                lhsT = w8_t[:p, j * 2 * _SEG:(j + 1) * 2 * _SEG].rearrange(
                    "p (two m) -> p two m", two=2)
                rhs = g[:p, goff:goff + 2 * _H].rearrange(
                    "p (two h) -> p two h", two=2)
                st = j == 0
                sp = j == n2 - 1
                i2[0] += 1
                nc.tensor.matmul(ps2_a[:], lhsT, rhs[:, :, 0:512],
                                 start=st, stop=sp,
                                 perf_mode=mybir.MatmulPerfMode.DoubleRow)
                nc.tensor.matmul(ps2_b[:], lhsT, rhs[:, :, 512:_H],
                                 start=st, stop=sp,
                                 perf_mode=mybir.MatmulPerfMode.DoubleRow)

            def mm_p1(p, g):
                j = i2[0]
                lhsT = w8_t[:p, j * 2 * _SEG:j * 2 * _SEG + _SEG]
                st = j == 0
                sp = j == n2 - 1
                i2[0] += 1
                nc.tensor.matmul(ps2_a[:], lhsT, g[:p, 0:512],
                                 start=st, stop=sp)
                nc.tensor.matmul(ps2_b[:], lhsT, g[:p, 512:_H],
                                 start=st, stop=sp)

            oz = od = op = 0
            qsel = [0]

            def gather(out_ap, in_ap):
                # alternate the two DMA queues (Sync HWDGE / Pool SWDGE) so
                # per-queue issue latency and semaphore waits don't gate the
                # stream
                if qsel[0] % 2 == 0:
                    nc.sync.dma_start(out=out_ap, in_=in_ap)
                else:
                    nc.gpsimd.dma_start(out=out_ap, in_=in_ap)
                qsel[0] += 1

            with nc.allow_low_precision(reason="bf16/fp8 span sums; rel "
                                        "tolerance 2e-2"):
                for kind, p in spec:
                    if kind == "Z":
                        g = zpool.tile([128, 4 * _H], bf16, tag="gz")
                        gather(g[:p], xz[oz:oz + 4 * p].rearrange(
                            "(p k) h -> p (k h)", k=4, h=_H))
                        oz += 4 * p
                        for wi in range(2):
                            red = rpool.tile([128, _H], bf16, tag="rz")
                            base = 2 * wi * _H
                            nc.vector.tensor_add(
                                red[:p], g[:p, base:base + _H],
                                g[:p, base + _H:base + 2 * _H])
                            mm_pair(p, red)
                    elif kind == "D":
                        g = dpool.tile([128, 4 * _H], e3, tag="gd")
                        gather(g[:p], xd[od:od + 4 * p].rearrange(
                            "(p k) h -> p (k h)", k=4, h=_H))
                        od += 4 * p
                        for wi in range(2):
                            red = rpool.tile([128, _H], e3, tag="rd")
                            base = 2 * wi * _H
                            nc.vector.tensor_add(
                                red[:p], g[:p, base:base + _H],
                                g[:p, base + _H:base + 2 * _H])
                            mm_pair(p, red)
                    elif kind == "P":
                        g = ppool_g.tile([128, 4 * _H], e4, tag="gp")
                        gather(g[:p], xp[op:op + 4 * p].rearrange(
                            "(p k) h -> p (k h)", k=4, h=_H))
                        op += 4 * p
                        mm_dr(p, g, 0)
                        mm_dr(p, g, 2 * _H)
                    else:  # P1
                        g = ppool_g.tile([128, _H], e4, tag="gp1")
                        gather(g[:p], xp[op:op + p].rearrange(
                            "(p k) h -> p (k h)", k=1, h=_H))
                        op += p
                        mm_p1(p, g)

            o_t = cpool.tile([_SEG, 2 * _H], f32)
            nc.vector.tensor_copy(o_t[:, 0:512], ps_a[:])
            nc.scalar.copy(o_t[:, 512:_H], ps_b[:])
            nc.vector.tensor_copy(o_t[:, _H:_H + 512], ps2_a[:])
            nc.scalar.copy(o_t[:, _H + 512:2 * _H], ps2_b[:])
            nc.sync.dma_start(out=outx[:], in_=o_t[:])
    nc.compile()
    return nc


def _spans(entity_positions):
    ep = np.asarray(entity_positions).astype(np.int64)
    hs = np.clip(ep[:, 0], 0, _L - 1)
    he = np.maximum(hs, np.minimum(ep[:, 1], _L - 1))
    ts = np.clip(ep[:, 2], 0, _L - 1)
    te = np.maximum(ts, np.minimum(ep[:, 3], _L - 1))
    return hs, he, ts, te


def _plan(entity_positions):
    """Returns per-core window/row lists.

    Per core:
      zw: bf16 two-row windows (b, r0, dup, wv)
      dw: e3m4 two-row windows (b, r0, wv)
      pr: e4m3 single rows (b, r, wv01)   [wv01 is the 0/1 mask row]
    """
    hs, he, ts, te = _spans(entity_positions)

    runs = []
    usize = np.zeros(_B, np.int64)
    for b in range(_B):
        a0, a1, b0, b1 = hs[b], he[b], ts[b], te[b]
        if a0 > b0:
            a0, a1, b0, b1 = b0, b1, a0, a1
        if b0 <= a1 + 1:
            r = [(int(a0), int(max(a1, b1)))]
        else:
            r = [(int(a0), int(a1)), (int(b0), int(b1))]
        runs.append(r)
        usize[b] = sum(e - s + 1 for s, e in r)

    order = np.argsort(-usize, kind="stable")
    loads = np.zeros(_NCORES, np.int64)
    core_batches = [[] for _ in range(_NCORES)]
    for b in order:
        open_cores = [c for c in range(_NCORES) if len(core_batches[c]) < _BPC]
        c = min(open_cores, key=lambda i: loads[i])
        core_batches[c].append(int(b))
        loads[c] += usize[b]

    hcnt = he - hs + 1
    tcnt = te - ts + 1

    def wvec(b, lb, r, ones):
        v = np.zeros(_SEG, np.float32)
        if hs[b] <= r <= he[b]:
            v[lb] = np.float32(1.0 if ones else 1.0 / hcnt[b])
        if ts[b] <= r <= te[b]:
            v[_BPC + lb] = np.float32(1.0 if ones else 1.0 / tcnt[b])
        return v

    zw = [[] for _ in range(_NCORES)]
    dw = [[] for _ in range(_NCORES)]
    pr = [[] for _ in range(_NCORES)]
    for c in range(_NCORES):
        for lb, b in enumerate(core_batches[c]):
            for (s, e) in runs[b]:
                cuts = {s, e + 1}
                for v in (hs[b], he[b] + 1, ts[b], te[b] + 1):
                    if s < v <= e:
                        cuts.add(int(v))
                bounds = sorted(cuts)
                for ss, ee in zip(bounds[:-1], bounds[1:]):
                    ee -= 1  # inclusive
                    in_h = bool(hs[b] <= ss <= he[b])
                    in_t = bool(ts[b] <= ss <= te[b])
                    cands = ([int(hcnt[b])] if in_h else []) + \
                        ([int(tcnt[b])] if in_t else [])
                    mincnt = min(cands) if cands else 1
                    if mincnt >= _T2:
                        # whole subsegment row-wise into the P lane
                        wv1 = wvec(b, lb, ss, ones=True)
                        for r in range(ss, ee + 1):
                            pr[c].append((b, r, wv1))
                        continue
                    wv = wvec(b, lb, ss, ones=False)
                    fp8_ok = mincnt >= _TMIN
                    r = ss
                    while ee - r + 1 >= 2:
                        if fp8_ok:
                            dw[c].append((b, r, wv))
                        else:
                            zw[c].append((b, r, False, wv))
                        r += 2
                    if r <= ee:
                        if fp8_ok:
                            pr[c].append((b, r, wvec(b, lb, ss, ones=True)))
                        else:
                            zw[c].append((b, r, True, wv * 0.5))

    return core_batches, zw, dw, pr


def _pack(core_batches, zw, dw, pr, xb, xd8, xp8):
    """Build per-core xz/xd/xp streams and weight matrices."""
    import ml_dtypes

    nz = max(len(l) for l in zw)
    nd = max(len(l) for l in dw)
    npr = max(len(l) for l in pr)
    spec = _build_spec(nz, nd, npr)
    n1 = sum(_pairs_of(k)[0] for k, _ in spec)
    n2 = sum(_pairs_of(k)[1] for k, _ in spec)
    rz = sum(4 * s for k, s in spec if k == "Z")
    rd = sum(4 * s for k, s in spec if k == "D")
    rp = sum(_rows_per_part(k) * s for k, s in spec if k in ("P", "P1"))

    xz_mats, xd_mats, xp_mats, w_mats, w8_mats = [], [], [], [], []
    for c in range(_NCORES):
        xz_m = np.zeros((max(rz, 4), _H), ml_dtypes.bfloat16)
        xd_m = np.zeros((max(rd, 4), _H), ml_dtypes.float8_e3m4)
        xp_m = np.zeros((max(rp, 4), _H), ml_dtypes.float8_e4m3fn)
        wr = np.zeros((max(n1, 1), 128, _SEG), np.float32)
        wr8 = np.zeros((max(n2, 1), 128, 2 * _SEG), np.float32)
        zi = di = pi = 0
        oz = od = op = 0
        j1 = j2 = 0
        for kind, s in spec:
            if kind == "Z":
                for wi in range(2):
                    for p in range(s):
                        if zi < len(zw[c]):
                            b, r0, dup, wv = zw[c][zi]
                            zi += 1
                            xz_m[oz + 4 * p + 2 * wi] = xb[b, r0]
                            xz_m[oz + 4 * p + 2 * wi + 1] = \
                                xb[b, r0] if dup else xb[b, r0 + 1]
                            wr[j1 + wi, p] = wv
                oz += 4 * s
                j1 += 2
            elif kind == "D":
                for wi in range(2):
                    for p in range(s):
                        if di < len(dw[c]):
                            b, r0, wv = dw[c][di]
                            di += 1
                            xd_m[od + 4 * p + 2 * wi] = xd8[b, r0]
                            xd_m[od + 4 * p + 2 * wi + 1] = xd8[b, r0 + 1]
                            wr[j1 + wi, p] = wv
                od += 4 * s
                j1 += 2
            elif kind == "P":
                # DR group g: rows 4p+2g (A block), 4p+2g+1 (B block);
                # weights blocked [A 32 | B 32] at pair j2+g
                for g_ in range(2):
                    for half in range(2):
                        for p in range(s):
                            if pi < len(pr[c]):
                                b, r, wv1 = pr[c][pi]
                                pi += 1
                                xp_m[op + 4 * p + 2 * g_ + half] = xp8[b, r]
                                wr8[j2 + g_, p,
                                    half * _SEG:(half + 1) * _SEG] = wv1
                op += 4 * s
                j2 += 2
            else:  # P1
                for p in range(s):
                    if pi < len(pr[c]):
                        b, r, wv1 = pr[c][pi]
                        pi += 1
                        xp_m[op + p] = xp8[b, r]
                        wr8[j2, p, 0:_SEG] = wv1
                op += s
                j2 += 1
        xz_mats.append(xz_m)
        xd_mats.append(xd_m)
        xp_mats.append(xp_m)
        w_mats.append(np.ascontiguousarray(
            wr.transpose(1, 0, 2).reshape(128, -1)).astype(
                ml_dtypes.bfloat16))
        w8_mats.append(np.ascontiguousarray(
            wr8.transpose(1, 0, 2).reshape(128, -1)).astype(
                ml_dtypes.float8_e4m3fn))

    return spec, (xz_mats, xd_mats, xp_mats, w_mats, w8_mats), (nz, nd, npr)


def _run(sequence_output, entity_positions, trace=False, trace_cores=None):
    import ml_dtypes
    from concourse.bass_utils import run_bass_kernel_spmd

    x = np.asarray(sequence_output)
    xb = x.astype(ml_dtypes.bfloat16)
    xd8 = x.astype(ml_dtypes.float8_e3m4)
    xp8 = x.astype(ml_dtypes.float8_e4m3fn)
    core_batches, zw, dw, pr = _plan(entity_positions)
    spec, mats, key = _pack(core_batches, zw, dw, pr, xb, xd8, xp8)
    xz_mats, xd_mats, xp_mats, w_mats, w8_mats = mats

    if key not in _prog_cache:
        _prog_cache[key] = _build_program(*key)
    nc = _prog_cache[key]

    in_maps = []
    for c in range(_NCORES):
        in_maps.append({"xz": xz_mats[c], "xd": xd_mats[c], "xp": xp_mats[c],
                        "w": w_mats[c], "w8": w8_mats[c]})

    res = run_bass_kernel_spmd(
        nc, in_maps, list(range(_NCORES)), trace=trace,
        trace_cores=trace_cores,
    )

    hs, he, ts, te = _spans(entity_positions)
    hcnt = (he - hs + 1).astype(np.float32)
    tcnt = (te - ts + 1).astype(np.float32)

    head = np.zeros((_B, _H), np.float32)
    tail = np.zeros((_B, _H), np.float32)
    for c in range(_NCORES):
        ox = res.results[c]["outx"]
        o = ox[:, 0:_H]
        o2 = ox[:, _H:2 * _H]
        for lb, b in enumerate(core_batches[c]):
            head[b] = o[lb] + o2[lb] / hcnt[b]
            tail[b] = o[_BPC + lb] + o2[_BPC + lb] / tcnt[b]
    return (head, tail), res


def kernel(sequence_output, entity_positions):
    (head, tail), _ = _run(sequence_output, entity_positions)
    return head, tail


# revision 23
# speedup vs baseline: 1.0700x; 1.0700x over previous
"""Trainium2 Bass kernel for EntityMarker segment-reduce (span means).

Problem: sequence_output [128, 2048, 768] f32, entity_positions [128, 4] int.
For each batch b, compute the mean of sequence_output[b, s:e+1, :] for the
head span (cols 0,1) and tail span (cols 2,3), clamped like the reference.
Output: (head [128, 768], tail [128, 768]) f32.

v3.1 strategy (host-permuted bf16/fp8 triple stream, 8 cores):
  - Host computes clamped spans; per batch the union of the two spans is 1-2
    contiguous runs, split into subsegments of constant (head, tail)
    membership. Batches go to cores by greedy LPT on union size (16/core).
  - The host permutes the needed rows into the exact order the device
    consumes them, so the device does NO gather — every chunk is one plain
    2D DMA from a contiguous DRAM region. The DMA engines are elementwise
    (SBUF-write-side) bound, so bytes are minimized by SBUF dtype tiering
    on the min span count a row feeds (span-mean quantization error shrinks
    as 1/sqrt(count)):
      count < 16        -> bf16   (Z chunks)
      16 <= count < 64  -> fp8 e3m4 (D chunks)
      count >= 64       -> fp8 e4m3 (P chunks)
  - Chunk kinds (tile = [<=128 partitions, 4*768]):
      Z (bf16): partition = 2 two-row windows; 2 DVE adds (2x mode) ->
        bf16 reduced rows; 2 PE matmul pairs (bf16 1/count weights) -> PSUM1.
      D (e3m4): same shape; 2 DVE adds (1x) -> e3m4 reduced rows; 2 matmul
        pairs (bf16 lhsT x fp8 rhs runs at fp8 rate) -> PSUM1.
      P (e4m3): no DVE; 2 DoubleRow matmul pairs (0.5 cycles/row) contract
        256 raw rows each with 0/1 e4m3 per-row weights -> PSUM2.
      P1 (e4m3): single-row tail chunk, one plain pair -> PSUM2.
    Odd leftover bf16 rows are duplicated into a two-row window with halved
    weights; odd fp8 rows go to the P lane (per-row weights).
  - PSUM2 holds RAW per-segment sums; it is drained to a second output and
    the host folds final = out + out2 / count. This keeps P-lane weights
    exactly representable in e4m3.
  - The stream ends with a tiny P1 chunk so the only work after the last
    byte is one matmul pair + the PSUM drain.
  - The device program is UNIFORM across cores (SPMD); per-core shortfalls
    are padded with zero rows and zero weights.
"""

import os

import numpy as np

_B, _L, _H = 128, 2048, 768
_NCORES = 8
_BPC = _B // _NCORES  # batches per core
_SEG = 2 * _BPC       # segments per core: 16 head + 16 tail
_TMIN = int(os.environ.get("KERNEL_TMIN", "16"))   # min count for fp8
_T2 = int(os.environ.get("KERNEL_T2", "64"))       # min count for e4m3
_PSDMA = os.environ.get("KERNEL_PSDMA", "0") == "1"
_DBUFS = int(os.environ.get("KERNEL_DBUFS", "6"))
_PBUFS = int(os.environ.get("KERNEL_PBUFS", "6"))

_prog_cache = {}


def _mult16(n):
    return (n + 15) // 16 * 16


def _chunks_of(n_units, units_per_chunk):
    """Split n_units windows/rows into chunk partition-widths (mult of 16)."""
    full = 128 * units_per_chunk
    sizes = []
    left = n_units
    while left >= full:
        sizes.append(128)
        left -= full
    if left > 0:
        sizes.append(_mult16((left + units_per_chunk - 1) // units_per_chunk))
    return sizes


def _build_spec(nz, nd, np_rows):
    """Uniform chunk spec: list of (kind, partition_width).

    P/D interleaved (PE is the longest compute pole, feed it first and
    steadily), Z in the middle, ending with a 1-row-deep P1 chunk."""
    zc = _chunks_of(nz, 2)
    dc = _chunks_of(nd, 2)
    p_tail = min(np_rows, 128)
    pc = _chunks_of(np_rows - p_tail, 4)
    spec = []
    di, pi, zi = 0, 0, 0
    # round-robin: 3 P : 1 D : occasionally Z
    while di < len(dc) or pi < len(pc) or zi < len(zc):
        for _ in range(3):
            if pi < len(pc):
                spec.append(("P", pc[pi]))
                pi += 1
        if di < len(dc):
            spec.append(("D", dc[di]))
            di += 1
        if zi < len(zc):
            spec.append(("Z", zc[zi]))
            zi += 1
    spec.append(("P1", _mult16(p_tail)))
    return spec


def _pairs_of(kind):
    # (psum1 pairs, psum2 pairs) per chunk
    return {"Z": (2, 0), "D": (2, 0), "P": (0, 2), "P1": (0, 1)}[kind]


def _rows_per_part(kind):
    return {"Z": 4, "D": 4, "P": 4, "P1": 1}[kind]


def _build_program(nz, nd, np_rows):
    import concourse.mybir as mybir
    from concourse import bacc, tile

    f32 = mybir.dt.float32
    bf16 = mybir.dt.bfloat16
    e3 = mybir.dt.float8e3
    e4 = mybir.dt.float8e4

    spec = _build_spec(nz, nd, np_rows)
    n1 = sum(_pairs_of(k)[0] for k, _ in spec)
    n2 = sum(_pairs_of(k)[1] for k, _ in spec)
    rz = sum(4 * s for k, s in spec if k == "Z")
    rd = sum(4 * s for k, s in spec if k == "D")
    rp = sum(_rows_per_part(k) * s for k, s in spec if k in ("P", "P1"))

    nc = bacc.Bacc(None, target_bir_lowering=False)
    xz = nc.declare_dram_parameter("xz", [max(rz, 4), _H], bf16,
                                   isOutput=False)
    xd = nc.declare_dram_parameter("xd", [max(rd, 4), _H], e3,
                                   isOutput=False)
    xp = nc.declare_dram_parameter("xp", [max(rp, 4), _H], e4,
                                   isOutput=False)
    w = nc.declare_dram_parameter("w", [128, max(n1, 1) * _SEG], bf16,
                                  isOutput=False)
    # P-lane 0/1 weights: 64 cols per DR pair group-block (2x32), 32 for P1
    w8 = nc.declare_dram_parameter("w8", [128, max(n2, 1) * 2 * _SEG], e4,
                                   isOutput=False)
    # single fused output: cols [0:768] = weighted sums, [768:1536] = raw
    # P-lane sums (host divides by count and adds)
    outx = nc.declare_dram_parameter("outx", [_SEG, 2 * _H], f32,
                                     isOutput=True)

    with tile.TileContext(nc) as tc:
        with (
            tc.tile_pool(name="const", bufs=1) as cpool,
            tc.tile_pool(name="gz", bufs=2) as zpool,
            tc.tile_pool(name="gd", bufs=_DBUFS) as dpool,
            tc.tile_pool(name="gp", bufs=_PBUFS) as ppool_g,
            tc.tile_pool(name="red", bufs=8) as rpool,
            tc.tile_pool(name="psum", bufs=1, space="PSUM") as pspool,
        ):
            w_t = cpool.tile([128, max(n1, 1) * _SEG], bf16)
            nc.scalar.dma_start(out=w_t[:], in_=w[:])
            w8_t = cpool.tile([128, max(n2, 1) * 2 * _SEG], e4)
            nc.scalar.dma_start(out=w8_t[:], in_=w8[:])

            ps_a = pspool.tile([_SEG, 512], f32)
            ps_b = pspool.tile([_SEG, 256], f32)
            ps2_a = pspool.tile([_SEG, 512], f32)
            ps2_b = pspool.tile([_SEG, 256], f32)

            i1, i2 = [0], [0]

            def mm_pair(p, rhs):
                j = i1[0]
                lhsT = w_t[:p, j * _SEG:(j + 1) * _SEG]
                st = j == 0
                sp = j == n1 - 1
                i1[0] += 1
                nc.tensor.matmul(ps_a[:], lhsT, rhs[:p, 0:512],
                                 start=st, stop=sp)
                nc.tensor.matmul(ps_b[:], lhsT, rhs[:p, 512:_H],
                                 start=st, stop=sp)

            def mm_dr(p, g, goff):
                # DoubleRow pair: blocked APs [p, 2, N]; contracts 2*p rows
                j = i2[0]
                lhsT = w8_t[:p, j * 2 * _SEG:(j + 1) * 2 * _SEG].rearrange(
                    "p (two m) -> p two m", two=2)
                rhs = g[:p, goff:goff + 2 * _H].rearrange(
                    "p (two h) -> p two h", two=2)
                st = j == 0
                sp = j == n2 - 1
                i2[0] += 1
                nc.tensor.matmul(ps2_a[:], lhsT, rhs[:, :, 0:512],
                                 start=st, stop=sp,
                                 perf_mode=mybir.MatmulPerfMode.DoubleRow)
                nc.tensor.matmul(ps2_b[:], lhsT, rhs[:, :, 512:_H],
                                 start=st, stop=sp,
                                 perf_mode=mybir.MatmulPerfMode.DoubleRow)

            def mm_p1(p, g):
                j = i2[0]
                lhsT = w8_t[:p, j * 2 * _SEG:j * 2 * _SEG + _SEG]
                st = j == 0
                sp = j == n2 - 1
                i2[0] += 1
                nc.tensor.matmul(ps2_a[:], lhsT, g[:p, 0:512],
                                 start=st, stop=sp)
                nc.tensor.matmul(ps2_b[:], lhsT, g[:p, 512:_H],
                                 start=st, stop=sp)

            oz = od = op = 0
            qsel = [0]

            def gather(out_ap, in_ap):
                # all gathers on the Sync HWDGE queue (measured faster than
                # alternating with the Pool SWDGE queue)
                qsel[0] += 1
                nc.sync.dma_start(out=out_ap, in_=in_ap)

            with nc.allow_low_precision(reason="bf16/fp8 span sums; rel "
                                        "tolerance 2e-2"):
                for kind, p in spec:
                    if kind == "Z":
                        g = zpool.tile([128, 4 * _H], bf16, tag="gz")
                        gather(g[:p], xz[oz:oz + 4 * p].rearrange(
                            "(p k) h -> p (k h)", k=4, h=_H))
                        oz += 4 * p
                        for wi in range(2):
                            red = rpool.tile([128, _H], bf16, tag="rz")
                            base = 2 * wi * _H
                            nc.vector.tensor_add(
                                red[:p], g[:p, base:base + _H],
                                g[:p, base + _H:base + 2 * _H])
                            mm_pair(p, red)
                    elif kind == "D":
                        g = dpool.tile([128, 4 * _H], e3, tag="gd")
                        gather(g[:p], xd[od:od + 4 * p].rearrange(
                            "(p k) h -> p (k h)", k=4, h=_H))
                        od += 4 * p
                        for wi in range(2):
                            red = rpool.tile([128, _H], e3, tag="rd")
                            base = 2 * wi * _H
                            nc.vector.tensor_add(
                                red[:p], g[:p, base:base + _H],
                                g[:p, base + _H:base + 2 * _H])
                            mm_pair(p, red)
                    elif kind == "P":
                        g = ppool_g.tile([128, 4 * _H], e4, tag="gp")
                        gather(g[:p], xp[op:op + 4 * p].rearrange(
                            "(p k) h -> p (k h)", k=4, h=_H))
                        op += 4 * p
                        mm_dr(p, g, 0)
                        mm_dr(p, g, 2 * _H)
                    else:  # P1
                        g = ppool_g.tile([128, _H], e4, tag="gp1")
                        gather(g[:p], xp[op:op + p].rearrange(
                            "(p k) h -> p (k h)", k=1, h=_H))
                        op += p
                        mm_p1(p, g)

            o_t = cpool.tile([_SEG, 2 * _H], f32)
            nc.vector.tensor_copy(o_t[:, 0:512], ps_a[:])
            nc.scalar.copy(o_t[:, 512:_H], ps_b[:])
            nc.vector.tensor_copy(o_t[:, _H:_H + 512], ps2_a[:])
            nc.scalar.copy(o_t[:, _H + 512:2 * _H], ps2_b[:])
            nc.sync.dma_start(out=outx[:], in_=o_t[:])
    nc.compile()
    return nc


def _spans(entity_positions):
    ep = np.asarray(entity_positions).astype(np.int64)
    hs = np.clip(ep[:, 0], 0, _L - 1)
    he = np.maximum(hs, np.minimum(ep[:, 1], _L - 1))
    ts = np.clip(ep[:, 2], 0, _L - 1)
    te = np.maximum(ts, np.minimum(ep[:, 3], _L - 1))
    return hs, he, ts, te


def _plan(entity_positions):
    """Returns per-core window/row lists.

    Per core:
      zw: bf16 two-row windows (b, r0, dup, wv)
      dw: e3m4 two-row windows (b, r0, wv)
      pr: e4m3 single rows (b, r, wv01)   [wv01 is the 0/1 mask row]
    """
    hs, he, ts, te = _spans(entity_positions)

    runs = []
    usize = np.zeros(_B, np.int64)
    for b in range(_B):
        a0, a1, b0, b1 = hs[b], he[b], ts[b], te[b]
        if a0 > b0:
            a0, a1, b0, b1 = b0, b1, a0, a1
        if b0 <= a1 + 1:
            r = [(int(a0), int(max(a1, b1)))]
        else:
            r = [(int(a0), int(a1)), (int(b0), int(b1))]
        runs.append(r)
        usize[b] = sum(e - s + 1 for s, e in r)

    order = np.argsort(-usize, kind="stable")
    loads = np.zeros(_NCORES, np.int64)
    core_batches = [[] for _ in range(_NCORES)]
    for b in order:
        open_cores = [c for c in range(_NCORES) if len(core_batches[c]) < _BPC]
        c = min(open_cores, key=lambda i: loads[i])
        core_batches[c].append(int(b))
        loads[c] += usize[b]

    hcnt = he - hs + 1
    tcnt = te - ts + 1

    def wvec(b, lb, r, ones):
        v = np.zeros(_SEG, np.float32)
        if hs[b] <= r <= he[b]:
            v[lb] = np.float32(1.0 if ones else 1.0 / hcnt[b])
        if ts[b] <= r <= te[b]:
            v[_BPC + lb] = np.float32(1.0 if ones else 1.0 / tcnt[b])
        return v

    zw = [[] for _ in range(_NCORES)]
    dw = [[] for _ in range(_NCORES)]
    pr = [[] for _ in range(_NCORES)]
    for c in range(_NCORES):
        for lb, b in enumerate(core_batches[c]):
            for (s, e) in runs[b]:
                cuts = {s, e + 1}
                for v in (hs[b], he[b] + 1, ts[b], te[b] + 1):
                    if s < v <= e:
                        cuts.add(int(v))
                bounds = sorted(cuts)
                for ss, ee in zip(bounds[:-1], bounds[1:]):
                    ee -= 1  # inclusive
                    in_h = bool(hs[b] <= ss <= he[b])
                    in_t = bool(ts[b] <= ss <= te[b])
                    cands = ([int(hcnt[b])] if in_h else []) + \
                        ([int(tcnt[b])] if in_t else [])
                    mincnt = min(cands) if cands else 1
                    if mincnt >= _T2:
                        # whole subsegment row-wise into the P lane
                        wv1 = wvec(b, lb, ss, ones=True)
                        for r in range(ss, ee + 1):
                            pr[c].append((b, r, wv1))
                        continue
                    wv = wvec(b, lb, ss, ones=False)
                    fp8_ok = mincnt >= _TMIN
                    r = ss
                    while ee - r + 1 >= 2:
                        if fp8_ok:
                            dw[c].append((b, r, wv))
                        else:
                            zw[c].append((b, r, False, wv))
                        r += 2
                    if r <= ee:
                        if fp8_ok:
                            pr[c].append((b, r, wvec(b, lb, ss, ones=True)))
                        else:
                            zw[c].append((b, r, True, wv * 0.5))

    return core_batches, zw, dw, pr


def _pack(core_batches, zw, dw, pr, xb, xd8, xp8):
    """Build per-core xz/xd/xp streams and weight matrices."""
    import ml_dtypes

    nz = max(len(l) for l in zw)
    nd = max(len(l) for l in dw)
    npr = max(len(l) for l in pr)
    spec = _build_spec(nz, nd, npr)
    n1 = sum(_pairs_of(k)[0] for k, _ in spec)
    n2 = sum(_pairs_of(k)[1] for k, _ in spec)
    rz = sum(4 * s for k, s in spec if k == "Z")
    rd = sum(4 * s for k, s in spec if k == "D")
    rp = sum(_rows_per_part(k) * s for k, s in spec if k in ("P", "P1"))

    xz_mats, xd_mats, xp_mats, w_mats, w8_mats = [], [], [], [], []
    for c in range(_NCORES):
        xz_m = np.zeros((max(rz, 4), _H), ml_dtypes.bfloat16)
        xd_m = np.zeros((max(rd, 4), _H), ml_dtypes.float8_e3m4)
        xp_m = np.zeros((max(rp, 4), _H), ml_dtypes.float8_e4m3fn)
        wr = np.zeros((max(n1, 1), 128, _SEG), np.float32)
        wr8 = np.zeros((max(n2, 1), 128, 2 * _SEG), np.float32)
        zi = di = pi = 0
        oz = od = op = 0
        j1 = j2 = 0
        for kind, s in spec:
            if kind == "Z":
                for wi in range(2):
                    for p in range(s):
                        if zi < len(zw[c]):
                            b, r0, dup, wv = zw[c][zi]
                            zi += 1
                            xz_m[oz + 4 * p + 2 * wi] = xb[b, r0]
                            xz_m[oz + 4 * p + 2 * wi + 1] = \
                                xb[b, r0] if dup else xb[b, r0 + 1]
                            wr[j1 + wi, p] = wv
                oz += 4 * s
                j1 += 2
            elif kind == "D":
                for wi in range(2):
                    for p in range(s):
                        if di < len(dw[c]):
                            b, r0, wv = dw[c][di]
                            di += 1
                            xd_m[od + 4 * p + 2 * wi] = xd8[b, r0]
                            xd_m[od + 4 * p + 2 * wi + 1] = xd8[b, r0 + 1]
                            wr[j1 + wi, p] = wv
                od += 4 * s
                j1 += 2
            elif kind == "P":
                # DR group g: rows 4p+2g (A block), 4p+2g+1 (B block);
                # weights blocked [A 32 | B 32] at pair j2+g
                for g_ in range(2):
                    for half in range(2):
                        for p in range(s):
                            if pi < len(pr[c]):
                                b, r, wv1 = pr[c][pi]
                                pi += 1
                                xp_m[op + 4 * p + 2 * g_ + half] = xp8[b, r]
                                wr8[j2 + g_, p,
                                    half * _SEG:(half + 1) * _SEG] = wv1
                op += 4 * s
                j2 += 2
            else:  # P1
                for p in range(s):
                    if pi < len(pr[c]):
                        b, r, wv1 = pr[c][pi]
                        pi += 1
                        xp_m[op + p] = xp8[b, r]
                        wr8[j2, p, 0:_SEG] = wv1
                op += s
                j2 += 1
        xz_mats.append(xz_m)
        xd_mats.append(xd_m)
        xp_mats.append(xp_m)
        w_mats.append(np.ascontiguousarray(
            wr.transpose(1, 0, 2).reshape(128, -1)).astype(
                ml_dtypes.bfloat16))
        w8_mats.append(np.ascontiguousarray(
            wr8.transpose(1, 0, 2).reshape(128, -1)).astype(
                ml_dtypes.float8_e4m3fn))

    return spec, (xz_mats, xd_mats, xp_mats, w_mats, w8_mats), (nz, nd, npr)


def _run(sequence_output, entity_positions, trace=False, trace_cores=None):
    import ml_dtypes
    from concourse.bass_utils import run_bass_kernel_spmd

    x = np.asarray(sequence_output)
    xb = x.astype(ml_dtypes.bfloat16)
    xd8 = x.astype(ml_dtypes.float8_e3m4)
    xp8 = x.astype(ml_dtypes.float8_e4m3fn)
    core_batches, zw, dw, pr = _plan(entity_positions)
    spec, mats, key = _pack(core_batches, zw, dw, pr, xb, xd8, xp8)
    xz_mats, xd_mats, xp_mats, w_mats, w8_mats = mats

    if key not in _prog_cache:
        _prog_cache[key] = _build_program(*key)
    nc = _prog_cache[key]

    in_maps = []
    for c in range(_NCORES):
        in_maps.append({"xz": xz_mats[c], "xd": xd_mats[c], "xp": xp_mats[c],
                        "w": w_mats[c], "w8": w8_mats[c]})

    res = run_bass_kernel_spmd(
        nc, in_maps, list(range(_NCORES)), trace=trace,
        trace_cores=trace_cores,
    )

    hs, he, ts, te = _spans(entity_positions)
    hcnt = (he - hs + 1).astype(np.float32)
    tcnt = (te - ts + 1).astype(np.float32)

    head = np.zeros((_B, _H), np.float32)
    tail = np.zeros((_B, _H), np.float32)
    for c in range(_NCORES):
        ox = res.results[c]["outx"]
        o = ox[:, 0:_H]
        o2 = ox[:, _H:2 * _H]
        for lb, b in enumerate(core_batches[c]):
            head[b] = o[lb] + o2[lb] / hcnt[b]
            tail[b] = o[_BPC + lb] + o2[_BPC + lb] / tcnt[b]
    return (head, tail), res


def kernel(sequence_output, entity_positions):
    (head, tail), _ = _run(sequence_output, entity_positions)
    return head, tail


# revision 27
# speedup vs baseline: 1.2540x; 1.1719x over previous
"""Trainium2 Bass kernel for EntityMarker segment-reduce (span means).

Problem: sequence_output [128, 2048, 768] f32, entity_positions [128, 4] int.
For each batch b, compute the mean of sequence_output[b, s:e+1, :] for the
head span (cols 0,1) and tail span (cols 2,3), clamped like the reference.
Output: (head [128, 768], tail [128, 768]) f32.

v3.1 strategy (host-permuted bf16/fp8 triple stream, 8 cores):
  - Host computes clamped spans; per batch the union of the two spans is 1-2
    contiguous runs, split into subsegments of constant (head, tail)
    membership. Batches go to cores by greedy LPT on union size (16/core).
  - The host permutes the needed rows into the exact order the device
    consumes them, so the device does NO gather — every chunk is one plain
    2D DMA from a contiguous DRAM region. The DMA engines are elementwise
    (SBUF-write-side) bound, so bytes are minimized by SBUF dtype tiering
    on the min span count a row feeds (span-mean quantization error shrinks
    as 1/sqrt(count)):
      count < 16        -> bf16   (Z chunks)
      16 <= count < 64  -> fp8 e3m4 (D chunks)
      count >= 64       -> fp8 e4m3 (P chunks)
  - Chunk kinds (tile = [<=128 partitions, 4*768]):
      Z (bf16): partition = 2 two-row windows; 2 DVE adds (2x mode) ->
        bf16 reduced rows; 2 PE matmul pairs (bf16 1/count weights) -> PSUM1.
      D (e3m4): same shape; 2 DVE adds (1x) -> e3m4 reduced rows; 2 matmul
        pairs (bf16 lhsT x fp8 rhs runs at fp8 rate) -> PSUM1.
      P (e4m3): no DVE; 2 DoubleRow matmul pairs (0.5 cycles/row) contract
        256 raw rows each with 0/1 e4m3 per-row weights -> PSUM2.
      P1 (e4m3): single-row tail chunk, one plain pair -> PSUM2.
    Odd leftover bf16 rows are duplicated into a two-row window with halved
    weights; odd fp8 rows go to the P lane (per-row weights).
  - PSUM2 holds RAW per-segment sums; it is drained to a second output and
    the host folds final = out + out2 / count. This keeps P-lane weights
    exactly representable in e4m3.
  - The stream ends with a tiny P1 chunk so the only work after the last
    byte is one matmul pair + the PSUM drain.
  - The device program is UNIFORM across cores (SPMD); per-core shortfalls
    are padded with zero rows and zero weights.
"""

import os

import numpy as np

_B, _L, _H = 128, 2048, 768
_NCORES = 8
_BPC = _B // _NCORES  # batches per core
_SEG = 2 * _BPC       # segments per core: 16 head + 16 tail
_TMIN = int(os.environ.get("KERNEL_TMIN", "16"))   # min count for fp8
_T2 = int(os.environ.get("KERNEL_T2", "64"))       # min count for e4m3
_PSDMA = os.environ.get("KERNEL_PSDMA", "0") == "1"
_DBUFS = int(os.environ.get("KERNEL_DBUFS", "12"))
_PBUFS = int(os.environ.get("KERNEL_PBUFS", "12"))

_prog_cache = {}


def _mult16(n):
    return (n + 15) // 16 * 16


def _chunks_of(n_units, units_per_chunk):
    """Split n_units windows/rows into chunk partition-widths (mult of 16)."""
    full = 128 * units_per_chunk
    sizes = []
    left = n_units
    while left >= full:
        sizes.append(128)
        left -= full
    if left > 0:
        sizes.append(_mult16((left + units_per_chunk - 1) // units_per_chunk))
    return sizes


def _build_spec(nz, nd, np_rows):
    """Uniform chunk spec: list of (kind, partition_width).

    P/D interleaved (PE is the longest compute pole, feed it first and
    steadily), Z in the middle, ending with a 1-row-deep P1 chunk."""
    zc = _chunks_of(nz, 2)
    dc = _chunks_of(nd, 2)
    p_tail = min(np_rows, 128)
    pc = _chunks_of(np_rows - p_tail, 4)
    spec = []
    di, pi, zi = 0, 0, 0
    # round-robin: 3 P : 1 D : occasionally Z
    while di < len(dc) or pi < len(pc) or zi < len(zc):
        for _ in range(3):
            if pi < len(pc):
                spec.append(("P", pc[pi]))
                pi += 1
        if di < len(dc):
            spec.append(("D", dc[di]))
            di += 1
        if zi < len(zc):
            spec.append(("Z", zc[zi]))
            zi += 1
    spec.append(("P1", _mult16(p_tail)))
    return spec


def _pairs_of(kind):
    # (psum1 pairs, psum2 pairs) per chunk
    return {"Z": (2, 0), "D": (2, 0), "P": (0, 2), "P1": (0, 1)}[kind]


def _rows_per_part(kind):
    return {"Z": 4, "D": 4, "P": 4, "P1": 1}[kind]


def _build_program(nz, nd, np_rows):
    import concourse.mybir as mybir
    from concourse import bacc, tile

    f32 = mybir.dt.float32
    bf16 = mybir.dt.bfloat16
    e3 = mybir.dt.float8e3
    e4 = mybir.dt.float8e4

    spec = _build_spec(nz, nd, np_rows)
    n1 = sum(_pairs_of(k)[0] for k, _ in spec)
    n2 = sum(_pairs_of(k)[1] for k, _ in spec)
    rz = sum(4 * s for k, s in spec if k == "Z")
    rd = sum(4 * s for k, s in spec if k == "D")
    rp = sum(_rows_per_part(k) * s for k, s in spec if k in ("P", "P1"))

    nc = bacc.Bacc(None, target_bir_lowering=False)
    xz = nc.declare_dram_parameter("xz", [max(rz, 4), _H], bf16,
                                   isOutput=False)
    xd = nc.declare_dram_parameter("xd", [max(rd, 4), _H], e3,
                                   isOutput=False)
    xp = nc.declare_dram_parameter("xp", [max(rp, 4), _H], e4,
                                   isOutput=False)
    w = nc.declare_dram_parameter("w", [128, max(n1, 1) * _SEG], bf16,
                                  isOutput=False)
    # P-lane 0/1 weights: 64 cols per DR pair group-block (2x32), 32 for P1
    w8 = nc.declare_dram_parameter("w8", [128, max(n2, 1) * 2 * _SEG], e4,
                                   isOutput=False)
    # single fused output: cols [0:768] = weighted sums, [768:1536] = raw
    # P-lane sums (host divides by count and adds)
    outx = nc.declare_dram_parameter("outx", [_SEG, 2 * _H], f32,
                                     isOutput=True)

    with tile.TileContext(nc) as tc:
        with (
            tc.tile_pool(name="const", bufs=1) as cpool,
            tc.tile_pool(name="gz", bufs=2) as zpool,
            tc.tile_pool(name="gd", bufs=_DBUFS) as dpool,
            tc.tile_pool(name="gp", bufs=_PBUFS) as ppool_g,
            tc.tile_pool(name="gp1", bufs=1) as p1pool,
            tc.tile_pool(name="red", bufs=8) as rpool,
            tc.tile_pool(name="psum", bufs=1, space="PSUM") as pspool,
        ):
            w_t = cpool.tile([128, max(n1, 1) * _SEG], bf16)
            nc.scalar.dma_start(out=w_t[:], in_=w[:])
            w8_t = cpool.tile([128, max(n2, 1) * 2 * _SEG], e4)
            nc.scalar.dma_start(out=w8_t[:], in_=w8[:])

            ps_a = pspool.tile([_SEG, 512], f32)
            ps_b = pspool.tile([_SEG, 256], f32)
            ps2_a = pspool.tile([_SEG, 512], f32)
            ps2_b = pspool.tile([_SEG, 256], f32)

            i1, i2 = [0], [0]

            def mm_pair(p, rhs):
                j = i1[0]
                lhsT = w_t[:p, j * _SEG:(j + 1) * _SEG]
                st = j == 0
                sp = j == n1 - 1
                i1[0] += 1
                nc.tensor.matmul(ps_a[:], lhsT, rhs[:p, 0:512],
                                 start=st, stop=sp)
                nc.tensor.matmul(ps_b[:], lhsT, rhs[:p, 512:_H],
                                 start=st, stop=sp)

            def mm_dr(p, g, goff):
                # DoubleRow pair: blocked APs [p, 2, N]; contracts 2*p rows
                j = i2[0]
                lhsT = w8_t[:p, j * 2 * _SEG:(j + 1) * 2 * _SEG].rearrange(
                    "p (two m) -> p two m", two=2)
                rhs = g[:p, goff:goff + 2 * _H].rearrange(
                    "p (two h) -> p two h", two=2)
                st = j == 0
                sp = j == n2 - 1
                i2[0] += 1
                nc.tensor.matmul(ps2_a[:], lhsT, rhs[:, :, 0:512],
                                 start=st, stop=sp,
                                 perf_mode=mybir.MatmulPerfMode.DoubleRow)
                nc.tensor.matmul(ps2_b[:], lhsT, rhs[:, :, 512:_H],
                                 start=st, stop=sp,
                                 perf_mode=mybir.MatmulPerfMode.DoubleRow)

            def mm_p1(p, g):
                j = i2[0]
                lhsT = w8_t[:p, j * 2 * _SEG:j * 2 * _SEG + _SEG]
                st = j == 0
                sp = j == n2 - 1
                i2[0] += 1
                nc.tensor.matmul(ps2_a[:], lhsT, g[:p, 0:512],
                                 start=st, stop=sp)
                nc.tensor.matmul(ps2_b[:], lhsT, g[:p, 512:_H],
                                 start=st, stop=sp)

            oz = od = op = 0
            qsel = [0]

            def gather(out_ap, in_ap):
                # first chunks ride the Scalar queue (its sequencer finishes
                # boot ~4us before Sync's, so the stream starts earlier);
                # the rest go on the Sync HWDGE queue (alternating with the
                # Pool SWDGE queue measured slower)
                q = qsel[0]
                qsel[0] += 1
                if q < 3:
                    nc.scalar.dma_start(out=out_ap, in_=in_ap)
                else:
                    nc.sync.dma_start(out=out_ap, in_=in_ap)

            with nc.allow_low_precision(reason="bf16/fp8 span sums; rel "
                                        "tolerance 2e-2"):
                for kind, p in spec:
                    if kind == "Z":
                        g = zpool.tile([128, 4 * _H], bf16, tag="gz")
                        gather(g[:p], xz[oz:oz + 4 * p].rearrange(
                            "(p k) h -> p (k h)", k=4, h=_H))
                        oz += 4 * p
                        for wi in range(2):
                            red = rpool.tile([128, _H], bf16, tag="rz")
                            base = 2 * wi * _H
                            nc.vector.tensor_add(
                                red[:p], g[:p, base:base + _H],
                                g[:p, base + _H:base + 2 * _H])
                            mm_pair(p, red)
                    elif kind == "D":
                        g = dpool.tile([128, 4 * _H], e3, tag="gd")
                        gather(g[:p], xd[od:od + 4 * p].rearrange(
                            "(p k) h -> p (k h)", k=4, h=_H))
                        od += 4 * p
                        for wi in range(2):
                            red = rpool.tile([128, _H], e3, tag="rd")
                            base = 2 * wi * _H
                            nc.vector.tensor_add(
                                red[:p], g[:p, base:base + _H],
                                g[:p, base + _H:base + 2 * _H])
                            mm_pair(p, red)
                    elif kind == "P":
                        g = ppool_g.tile([128, 4 * _H], e4, tag="gp")
                        gather(g[:p], xp[op:op + 4 * p].rearrange(
                            "(p k) h -> p (k h)", k=4, h=_H))
                        op += 4 * p
                        mm_dr(p, g, 0)
                        mm_dr(p, g, 2 * _H)
                    else:  # P1
                        g = p1pool.tile([128, _H], e4, tag="gp1")
                        gather(g[:p], xp[op:op + p].rearrange(
                            "(p k) h -> p (k h)", k=1, h=_H))
                        op += p
                        mm_p1(p, g)

            o_t = cpool.tile([_SEG, 2 * _H], f32)
            nc.vector.tensor_copy(o_t[:, 0:512], ps_a[:])
            nc.scalar.copy(o_t[:, 512:_H], ps_b[:])
            nc.vector.tensor_copy(o_t[:, _H:_H + 512], ps2_a[:])
            nc.scalar.copy(o_t[:, _H + 512:2 * _H], ps2_b[:])
            nc.sync.dma_start(out=outx[:], in_=o_t[:])
    nc.compile()
    return nc


def _spans(entity_positions):
    ep = np.asarray(entity_positions).astype(np.int64)
    hs = np.clip(ep[:, 0], 0, _L - 1)
    he = np.maximum(hs, np.minimum(ep[:, 1], _L - 1))
    ts = np.clip(ep[:, 2], 0, _L - 1)
    te = np.maximum(ts, np.minimum(ep[:, 3], _L - 1))
    return hs, he, ts, te


def _plan(entity_positions):
    """Returns per-core window/row lists.

    Per core:
      zw: bf16 two-row windows (b, r0, dup, wv)
      dw: e3m4 two-row windows (b, r0, wv)
      pr: e4m3 single rows (b, r, wv01)   [wv01 is the 0/1 mask row]
    """
    hs, he, ts, te = _spans(entity_positions)

    runs = []
    usize = np.zeros(_B, np.int64)
    for b in range(_B):
        a0, a1, b0, b1 = hs[b], he[b], ts[b], te[b]
        if a0 > b0:
            a0, a1, b0, b1 = b0, b1, a0, a1
        if b0 <= a1 + 1:
            r = [(int(a0), int(max(a1, b1)))]
        else:
            r = [(int(a0), int(a1)), (int(b0), int(b1))]
        runs.append(r)
        usize[b] = sum(e - s + 1 for s, e in r)

    order = np.argsort(-usize, kind="stable")
    loads = np.zeros(_NCORES, np.int64)
    core_batches = [[] for _ in range(_NCORES)]
    for b in order:
        open_cores = [c for c in range(_NCORES) if len(core_batches[c]) < _BPC]
        c = min(open_cores, key=lambda i: loads[i])
        core_batches[c].append(int(b))
        loads[c] += usize[b]

    hcnt = he - hs + 1
    tcnt = te - ts + 1

    def wvec(b, lb, r, ones):
        v = np.zeros(_SEG, np.float32)
        if hs[b] <= r <= he[b]:
            v[lb] = np.float32(1.0 if ones else 1.0 / hcnt[b])
        if ts[b] <= r <= te[b]:
            v[_BPC + lb] = np.float32(1.0 if ones else 1.0 / tcnt[b])
        return v

    zw = [[] for _ in range(_NCORES)]
    dw = [[] for _ in range(_NCORES)]
    pr = [[] for _ in range(_NCORES)]
    for c in range(_NCORES):
        for lb, b in enumerate(core_batches[c]):
            for (s, e) in runs[b]:
                cuts = {s, e + 1}
                for v in (hs[b], he[b] + 1, ts[b], te[b] + 1):
                    if s < v <= e:
                        cuts.add(int(v))
                bounds = sorted(cuts)
                for ss, ee in zip(bounds[:-1], bounds[1:]):
                    ee -= 1  # inclusive
                    in_h = bool(hs[b] <= ss <= he[b])
                    in_t = bool(ts[b] <= ss <= te[b])
                    cands = ([int(hcnt[b])] if in_h else []) + \
                        ([int(tcnt[b])] if in_t else [])
                    mincnt = min(cands) if cands else 1
                    if mincnt >= _T2:
                        # whole subsegment row-wise into the P lane
                        wv1 = wvec(b, lb, ss, ones=True)
                        for r in range(ss, ee + 1):
                            pr[c].append((b, r, wv1))
                        continue
                    wv = wvec(b, lb, ss, ones=False)
                    fp8_ok = mincnt >= _TMIN
                    r = ss
                    while ee - r + 1 >= 2:
                        if fp8_ok:
                            dw[c].append((b, r, wv))
                        else:
                            zw[c].append((b, r, False, wv))
                        r += 2
                    if r <= ee:
                        if fp8_ok:
                            pr[c].append((b, r, wvec(b, lb, ss, ones=True)))
                        else:
                            zw[c].append((b, r, True, wv * 0.5))

    return core_batches, zw, dw, pr


def _pack(core_batches, zw, dw, pr, xb, xd8, xp8):
    """Build per-core xz/xd/xp streams and weight matrices."""
    import ml_dtypes

    nz = max(len(l) for l in zw)
    nd = max(len(l) for l in dw)
    npr = max(len(l) for l in pr)
    spec = _build_spec(nz, nd, npr)
    n1 = sum(_pairs_of(k)[0] for k, _ in spec)
    n2 = sum(_pairs_of(k)[1] for k, _ in spec)
    rz = sum(4 * s for k, s in spec if k == "Z")
    rd = sum(4 * s for k, s in spec if k == "D")
    rp = sum(_rows_per_part(k) * s for k, s in spec if k in ("P", "P1"))

    xz_mats, xd_mats, xp_mats, w_mats, w8_mats = [], [], [], [], []
    for c in range(_NCORES):
        xz_m = np.zeros((max(rz, 4), _H), ml_dtypes.bfloat16)
        xd_m = np.zeros((max(rd, 4), _H), ml_dtypes.float8_e3m4)
        xp_m = np.zeros((max(rp, 4), _H), ml_dtypes.float8_e4m3fn)
        wr = np.zeros((max(n1, 1), 128, _SEG), np.float32)
        wr8 = np.zeros((max(n2, 1), 128, 2 * _SEG), np.float32)
        zi = di = pi = 0
        oz = od = op = 0
        j1 = j2 = 0
        for kind, s in spec:
            if kind == "Z":
                for wi in range(2):
                    for p in range(s):
                        if zi < len(zw[c]):
                            b, r0, dup, wv = zw[c][zi]
                            zi += 1
                            xz_m[oz + 4 * p + 2 * wi] = xb[b, r0]
                            xz_m[oz + 4 * p + 2 * wi + 1] = \
                                xb[b, r0] if dup else xb[b, r0 + 1]
                            wr[j1 + wi, p] = wv
                oz += 4 * s
                j1 += 2
            elif kind == "D":
                for wi in range(2):
                    for p in range(s):
                        if di < len(dw[c]):
                            b, r0, wv = dw[c][di]
                            di += 1
                            xd_m[od + 4 * p + 2 * wi] = xd8[b, r0]
                            xd_m[od + 4 * p + 2 * wi + 1] = xd8[b, r0 + 1]
                            wr[j1 + wi, p] = wv
                od += 4 * s
                j1 += 2
            elif kind == "P":
                # DR group g: rows 4p+2g (A block), 4p+2g+1 (B block);
                # weights blocked [A 32 | B 32] at pair j2+g
                for g_ in range(2):
                    for half in range(2):
                        for p in range(s):
                            if pi < len(pr[c]):
                                b, r, wv1 = pr[c][pi]
                                pi += 1
                                xp_m[op + 4 * p + 2 * g_ + half] = xp8[b, r]
                                wr8[j2 + g_, p,
                                    half * _SEG:(half + 1) * _SEG] = wv1
                op += 4 * s
                j2 += 2
            else:  # P1
                for p in range(s):
                    if pi < len(pr[c]):
                        b, r, wv1 = pr[c][pi]
                        pi += 1
                        xp_m[op + p] = xp8[b, r]
                        wr8[j2, p, 0:_SEG] = wv1
                op += s
                j2 += 1
        xz_mats.append(xz_m)
        xd_mats.append(xd_m)
        xp_mats.append(xp_m)
        w_mats.append(np.ascontiguousarray(
            wr.transpose(1, 0, 2).reshape(128, -1)).astype(
                ml_dtypes.bfloat16))
        w8_mats.append(np.ascontiguousarray(
            wr8.transpose(1, 0, 2).reshape(128, -1)).astype(
                ml_dtypes.float8_e4m3fn))

    return spec, (xz_mats, xd_mats, xp_mats, w_mats, w8_mats), (nz, nd, npr)


def _run(sequence_output, entity_positions, trace=False, trace_cores=None):
    import ml_dtypes
    from concourse.bass_utils import run_bass_kernel_spmd

    x = np.asarray(sequence_output)
    xb = x.astype(ml_dtypes.bfloat16)
    xd8 = x.astype(ml_dtypes.float8_e3m4)
    xp8 = x.astype(ml_dtypes.float8_e4m3fn)
    core_batches, zw, dw, pr = _plan(entity_positions)
    spec, mats, key = _pack(core_batches, zw, dw, pr, xb, xd8, xp8)
    xz_mats, xd_mats, xp_mats, w_mats, w8_mats = mats

    if key not in _prog_cache:
        _prog_cache[key] = _build_program(*key)
    nc = _prog_cache[key]

    in_maps = []
    for c in range(_NCORES):
        in_maps.append({"xz": xz_mats[c], "xd": xd_mats[c], "xp": xp_mats[c],
                        "w": w_mats[c], "w8": w8_mats[c]})

    res = run_bass_kernel_spmd(
        nc, in_maps, list(range(_NCORES)), trace=trace,
        trace_cores=trace_cores,
    )

    hs, he, ts, te = _spans(entity_positions)
    hcnt = (he - hs + 1).astype(np.float32)
    tcnt = (te - ts + 1).astype(np.float32)

    head = np.zeros((_B, _H), np.float32)
    tail = np.zeros((_B, _H), np.float32)
    for c in range(_NCORES):
        ox = res.results[c]["outx"]
        o = ox[:, 0:_H]
        o2 = ox[:, _H:2 * _H]
        for lb, b in enumerate(core_batches[c]):
            head[b] = o[lb] + o2[lb] / hcnt[b]
            tail[b] = o[_BPC + lb] + o2[_BPC + lb] / tcnt[b]
    return (head, tail), res


def kernel(sequence_output, entity_positions):
    (head, tail), _ = _run(sequence_output, entity_positions)
    return head, tail
